# revision 1
# baseline (speedup 1.0000x reference)
"""Trainium2 Bass kernel for a transformer decoder block (self-attn + cross-attn + FFN).

Sharding: pure data-parallel over batch B=128 across 8 NeuronCores (16 batches/core).
Per-core kernel works feature-major ([d_model, tokens]) so every matmul contracts
over the partition axis with no runtime transposes:
  - LN gains/biases folded into the following weight matrices on host (exact).
  - Attention computed transposed: scores[s,t] = k.T @ q with k,q feature-major;
    softmax sum over partitions via ones-matmul; causal mask applied
    multiplicatively on exp() (exact); the 1/sum division is folded into the
    PV epilogue via a DMA partition-broadcast.
  - LN stats via ones-matmuls in float32r; rstd = exp(-0.5*ln(var+eps)) keeps
    ScalarE in one activation-table set.
"""
import numpy as np
import ml_dtypes

import concourse.bass as bass
import concourse.bacc as bacc
import concourse.hw_specs as _hw_specs

_gat_orig = _hw_specs.get_activation_tables


def _gat_one_set(arch):
    tables = _gat_orig(arch)
    AF_ = mybir.ActivationFunctionType
    for name, funcs in tables.items():
        if name != "natural_log_exp_and_others":
            funcs.discard(AF_.Exp)
            funcs.discard(AF_.Ln)
    return tables


bacc.get_activation_tables = _gat_one_set
import concourse.mybir as mybir
import concourse.tile as tile
from concourse.bass_utils import run_bass_kernel_spmd

FP32 = mybir.dt.float32
F32R = mybir.dt.float32r
BF16 = mybir.dt.bfloat16
AF = mybir.ActivationFunctionType
OP = mybir.AluOpType
bf16 = ml_dtypes.bfloat16

B, T, D, H, DH, F = 128, 256, 384, 6, 64, 4 * 384
NCORES = 8
BLOC = B // NCORES          # 16 batches per core
PAIRS = BLOC // 2           # 8 batch-pairs per core
TP = 2 * T                  # 512 tokens per pair
C = D // 128                # 3 feature chunks
FM = F // 128               # 12 ffn-hidden chunks
EPS = 1e-5


def _dram_bcast_ap(dram_tile, nparts, ncols):
    """AP reading a [1, ncols] DRAM scratch broadcast to [nparts, ncols]."""
    return bass.AP(
        tensor=dram_tile.tensor,
        offset=dram_tile.offset,
        ap=[[0, nparts], [1, ncols]],
    )


def _build_nc(n_pairs=PAIRS, reps=1):
    nc = bacc.Bacc("TRN2", target_bir_lowering=False)

    x_d = nc.dram_tensor("x", [n_pairs, C, 128, TP], FP32, kind="ExternalInput")
    enc_d = nc.dram_tensor("enc", [n_pairs, C, 128, TP], BF16, kind="ExternalInput")
    y_d = nc.dram_tensor("y", [n_pairs, C, 128, TP], FP32, kind="ExternalOutput")

    wnames = ["wq1", "wk1", "wv1", "wp1", "wq2", "wk2", "wv2", "wp2"]
    w_d = {n: nc.dram_tensor(n, [C, 128, D], BF16, kind="ExternalInput") for n in wnames}
    w_d["wf1"] = nc.dram_tensor("wf1", [C, 128, F], BF16, kind="ExternalInput")
    w_d["wf2"] = nc.dram_tensor("wf2", [FM, 128, D], BF16, kind="ExternalInput")

    bnames = ["bq1", "bk1", "bv1", "bp1", "bq2", "bp2", "bf2"]
    b_d = {n: nc.dram_tensor(n, [128, C], FP32, kind="ExternalInput") for n in bnames}
    b_d["bf1"] = nc.dram_tensor("bf1", [128, FM], FP32, kind="ExternalInput")

    m_d = {n: nc.dram_tensor(n, [128, TP], BF16, kind="ExternalInput")
           for n in ["mask0", "mask1"]}

    with tile.TileContext(nc) as tc:
        with (
            tc.tile_pool(name="wp", bufs=1) as wp,
            tc.tile_pool(name="p_x", bufs=18) as p_x,
            tc.tile_pool(name="p_xh", bufs=10) as p_xh,
            tc.tile_pool(name="p_qk", bufs=7) as p_qk,
            tc.tile_pool(name="p_vt", bufs=8) as p_vt,
            tc.tile_pool(name="p_exp", bufs=8) as p_exp,
            tc.tile_pool(name="p_bc", bufs=3) as p_bc,
            tc.tile_pool(name="p_h", bufs=13) as p_h,
            tc.tile_pool(name="p_t", bufs=3) as p_t,
            tc.tile_pool(name="ps_mm", bufs=2, space="PSUM") as ps_mm,
            tc.tile_pool(name="ps_sc", bufs=4, space="PSUM") as ps_sc,
            tc.tile_pool(name="ps_st", bufs=2, space="PSUM") as ps_st,
        ):
            # ---- static weights/consts ----
            w_sb = {}
            dma_engines = [nc.sync, nc.gpsimd, nc.scalar]
            w_i = 0
            for n, d in w_d.items():
                cols = d.shape[2]
                ws = [wp.tile([128, cols], BF16, name=f"w_{n}_{c}", tag=f"w_{n}_{c}")
                      for c in range(d.shape[0])]
                for c, t_ in enumerate(ws):
                    dma_engines[w_i % len(dma_engines)].dma_start(out=t_[:], in_=d[c])
                    w_i += 1
                w_sb[n] = ws
            b_sb = {}
            for n, d in b_d.items():
                t_ = wp.tile([128, d.shape[1]], FP32, name=f"b_{n}", tag=f"b_{n}")
                nc.sync.dma_start(out=t_[:], in_=d[:, :])
                b_sb[n] = t_
            mask_sb = {}
            for n, d in m_d.items():
                t_ = wp.tile([128, TP], BF16, name=f"m_{n}", tag=f"m_{n}")
                nc.sync.dma_start(out=t_[:], in_=d[:, :])
                mask_sb[n] = t_
            ones_b = wp.tile([128, 128], BF16, name="ones_b", tag="ones_b")
            nc.vector.memset(ones_b[:], 1.0)
            eps_t = wp.tile([128, 1], FP32, name="eps", tag="eps")
            nc.vector.memset(eps_t[:], EPS)

            def layernorm(x_t):
                """x_t: list of C fp32 [128,TP] tiles -> list of C bf16 [128,TP]."""
                sum_ps = ps_st.tile([128, TP], FP32, name="st_sum", tag="stats")
                sq_ps = ps_st.tile([128, TP], FP32, name="st_sq", tag="stats")
                for c in range(C):
                    xb = p_t.tile([128, TP], BF16, name="xbf", tag="xbf", bufs=6)
                    nc.gpsimd.tensor_copy(xb[:], x_t[c][:])
                    x2 = p_t.tile([128, TP], BF16, name="x2", tag="x2", bufs=6)
                    nc.vector.tensor_mul(x2[:], xb[:], xb[:])
                    nc.tensor.matmul(sum_ps[:], ones_b[:], xb[:],
                                     start=(c == 0), stop=(c == C - 1))
                    nc.tensor.matmul(sq_ps[:], ones_b[:], x2[:],
                                     start=(c == 0), stop=(c == C - 1))
                # stats, already broadcast across partitions: [128, TP]
                negex = p_t.tile([128, TP], FP32, name="negex", tag="negex", bufs=3)
                nc.scalar.mul(negex[:], sum_ps[:], -1.0 / D)
                mu2 = p_t.tile([128, TP], FP32, name="mu2", tag="var", bufs=3)
                nc.vector.tensor_mul(mu2[:], negex[:], negex[:])
                ex2 = p_t.tile([128, TP], FP32, name="ex2", tag="var", bufs=3)
                nc.scalar.mul(ex2[:], sq_ps[:], 1.0 / D)
                var = p_t.tile([128, TP], FP32, name="var", tag="var", bufs=3)
                nc.vector.tensor_sub(var[:], ex2[:], mu2[:])
                lnv = p_t.tile([128, TP], FP32, name="lnv", tag="lnv", bufs=2)
                nc.scalar.activation(lnv[:], var[:], AF.Ln, bias=eps_t[:], scale=1.0)
                rstd = p_t.tile([128, TP], FP32, name="rstd", tag="lnt2", bufs=6)
                nc.scalar.activation(rstd[:], lnv[:], AF.Exp, bias=0.0, scale=-0.5)
                nmr = p_t.tile([128, TP], FP32, name="nmr", tag="lnt2", bufs=6)
                nc.vector.tensor_mul(nmr[:], negex[:], rstd[:])
                xh = []
                for c in range(C):
                    t1 = p_t.tile([128, TP], FP32, name="lnt1", tag="lnt3", bufs=4)
                    nc.vector.tensor_mul(t1[:], x_t[c][:], rstd[:])
                    xc = p_xh.tile([128, TP], BF16, name="xh", tag="xh")
                    nc.vector.tensor_add(xc[:], t1[:], nmr[:])
                    xh.append(xc)
                return xh

            def project(src_t, w, bias, n_out, act=AF.Identity, tag="q"):
                """out[m] = act(w[:,m-slice].T @ src + bias[:,m]); bf16 tiles [128,TP]."""
                outs = []
                for m in range(n_out):
                    ps = ps_mm.tile([128, TP], FP32, name="mm", tag="mm")
                    for c in range(C):
                        nc.tensor.matmul(ps[:], w[c][:, 128 * m:128 * (m + 1)],
                                         src_t[c][:], start=(c == 0), stop=(c == C - 1))
                    o = (p_h if tag == "h" else p_qk).tile([128, TP], BF16, name=tag, tag=tag)
                    if tag == "k":
                        if bias is not None:
                            nc.vector.tensor_scalar_add(o[:], ps[:], bias[:, m:m + 1])
                        else:
                            nc.vector.tensor_copy(o[:], ps[:])
                    elif bias is not None:
                        nc.scalar.activation(o[:], ps[:], act,
                                             bias=bias[:, m:m + 1], scale=1.0)
                    else:
                        nc.scalar.copy(o[:], ps[:])
                    outs.append(o)
                return outs

            def attention(q_src, kv_src, wq, bq, wk, bk, wv, bv, causal):
                """Returns list of C bf16 [128,TP] tiles (concat-head attn output)."""
                q_sb = project(q_src, w_sb[wq], b_sb.get(bq), C, tag="q")
                k_sb = project(kv_src, w_sb[wk], b_sb.get(bk), C, tag="k")
                attn = [p_qk.tile([128, TP], BF16, name="attn", tag="attn") for _ in range(C)]
                for b in range(2):
                    ob = b * T
                    # v^T for this batch: [128s, 384] per s-tile
                    vt_sb = []
                    for st in range(2):
                        vps = ps_mm.tile([128, D], FP32, name="vtp", tag="mm")
                        for c in range(C):
                            nc.tensor.matmul(
                                vps[:], kv_src[c][:, ob + 128 * st:ob + 128 * (st + 1)],
                                w_sb[wv][c][:], start=(c == 0), stop=(c == C - 1))
                        vt = p_vt.tile([128, D], BF16, name="vt", tag="vt")
                        nc.scalar.copy(vt[:], vps[:])
                        vt_sb.append(vt)
                    # scores + exp (+ causal mask), head-pair j packs 2 heads
                    exp_sb = [p_exp.tile([128, H * T], BF16, name="exp", tag="exp") for _ in range(2)]
                    for st in range(2):
                        for h in range(H):
                            j, e = h // 2, h % 2
                            pl = slice(64 * e, 64 * (e + 1))
                            sc = ps_sc.tile([128, T], FP32, name="sc", tag="scpv")
                            nc.tensor.matmul(
                                sc[:],
                                k_sb[j][pl, ob + 128 * st:ob + 128 * (st + 1)],
                                q_sb[j][pl, ob:ob + T],
                                start=True, stop=True)
                            sl = exp_sb[st][:, 256 * h:256 * (h + 1)]
                            nc.scalar.activation(sl, sc[:], AF.Exp, bias=0.0, scale=1.0)
                            if causal:
                                nc.vector.tensor_mul(
                                    sl, sl,
                                    mask_sb["mask0" if st == 0 else "mask1"][:, 0:T])
                    # softmax denominator: broadcast-sum then reciprocal
                    r_bc = p_bc.tile([128, H * T], BF16, name="r_bc", tag="r_bc")
                    for sl3 in range(C):
                        sm = ps_st.tile([128, TP], FP32, name="sm", tag="stats")
                        for st in range(2):
                            nc.tensor.matmul(
                                sm[:], ones_b[:],
                                exp_sb[st][:, 512 * sl3:512 * (sl3 + 1)],
                                start=(st == 0), stop=(st == 1))
                        with nc.allow_low_precision(reason="softmax 1/sum in bf16; scores O(1)"):
                            nc.vector.reciprocal(r_bc[:, 512 * sl3:512 * (sl3 + 1)], sm[:])
                    # PV: col-packed pairs; divide by sum; +v-bias; -> attn tiles
                    for j in range(C):
                        pv = ps_sc.tile([128, T], FP32, name="pv", tag="scpv")
                        for e in range(2):
                            h = 2 * j + e
                            for st in range(2):
                                nc.tensor.matmul(
                                    pv[64 * e:64 * (e + 1), :],
                                    vt_sb[st][:, 64 * h:64 * (h + 1)],
                                    exp_sb[st][:, 256 * h:256 * (h + 1)],
                                    start=(st == 0), stop=(st == 1),
                                    tile_position=(0, 64 * e))
                        for e in range(2):
                            h = 2 * j + e
                            pl = slice(64 * e, 64 * (e + 1))
                            nc.vector.tensor_mul(pv[pl, :], pv[pl, :],
                                                 r_bc[pl, 256 * h:256 * (h + 1)])
                        if bv is not None:
                            nc.vector.tensor_scalar_add(attn[j][:, ob:ob + T], pv[:],
                                                        b_sb[bv][:, j:j + 1])
                        else:
                            nc.vector.tensor_copy(attn[j][:, ob:ob + T], pv[:])
                return attn

            def proj_residual(x_t, src_t, w, bias, n_k):
                """x_new[m] = w[:,m].T @ src + bias[:,m] + x_old[m]."""
                x_new = []
                for m in range(C):
                    ps = ps_mm.tile([128, TP], FP32, name="mm", tag="mm")
                    for k in range(n_k):
                        nc.tensor.matmul(ps[:], w[k][:, 128 * m:128 * (m + 1)],
                                         src_t[k][:], start=(k == 0), stop=(k == n_k - 1))
                    xn = p_x.tile([128, TP], FP32, name="x", tag="x")
                    nc.vector.scalar_tensor_tensor(
                        out=xn[:], in0=ps[:], scalar=b_sb[bias][:, m:m + 1],
                        in1=x_t[m][:], op0=OP.add, op1=OP.add)
                    x_new.append(xn)
                return x_new

            # Two pairs are interleaved stage-by-stage so that one pair's
            # serial LN/softmax chains overlap the other pair's dense matmuls
            # (keeps PE busy and HAM warm across stage boundaries).
            for _rep in range(reps):
                for g in range(0, n_pairs, 2):
                    grp = [p for p in (g, g + 1) if p < n_pairs]
                    xs, xh, at, hh, ec = {}, {}, {}, {}, {}
                    for p in grp:
                        xs[p] = []
                        for c in range(C):
                            xc = p_x.tile([128, TP], FP32, name="x", tag="x")
                            nc.sync.dma_start(out=xc[:], in_=x_d[p, c])
                            xs[p].append(xc)
                    # stage 1: masked self-attention
                    for p in grp:
                        xh[p] = layernorm(xs[p])
                    for p in grp:
                        at[p] = attention(xh[p], xh[p], "wq1", "bq1", "wk1", "bk1",
                                          "wv1", "bv1", causal=True)
                    for p in grp:
                        xs[p] = proj_residual(xs[p], at[p], w_sb["wp1"], "bp1", C)
                    # stage 2: cross-attention (kv from raw encoder output)
                    for p in grp:
                        xh[p] = layernorm(xs[p])
                        ec[p] = []
                        for c in range(C):
                            e_ = p_xh.tile([128, TP], BF16, name="xh", tag="xh")
                            nc.sync.dma_start(out=e_[:], in_=enc_d[p, c])
                            ec[p].append(e_)
                    for p in grp:
                        at[p] = attention(xh[p], ec[p], "wq2", "bq2", "wk2", None,
                                          "wv2", None, causal=False)
                    for p in grp:
                        xs[p] = proj_residual(xs[p], at[p], w_sb["wp2"], "bp2", C)
                    # stage 3: FFN (per pair — avoids h-slot/psum slot cycles)
                    for p in grp:
                        xh[p] = layernorm(xs[p])
                    for p in grp:
                        hh[p] = project(xh[p], w_sb["wf1"], b_sb["bf1"], FM,
                                        act=AF.Relu, tag="h")
                        xs[p] = proj_residual(xs[p], hh[p], w_sb["wf2"], "bf2", FM)
                        for c in range(C):
                            nc.sync.dma_start(out=y_d[p, c], in_=xs[p][c][:])

    nc.compile()
    return nc


_NC = None


def _get_nc():
    global _NC
    if _NC is None:
        _NC = _build_nc()
    return _NC


def _prep_host(inputs):
    """Fold LN affine params into weights; pack everything into per-core maps."""
    f32 = np.float32
    inp = {k: np.asarray(v, f32) for k, v in inputs.items()}
    sc = f32(D) ** f32(-0.5)

    def fold_qkv(w, g, b, scale):
        # w: [H, D, DH] ; returns packed [C,128,D] weight + [128,C] bias
        we = w * g[None, :, None]
        be = np.einsum("c,hcd->hd", b, w)
        if scale is not None:
            we = we * scale
            be = be * scale
        W = we.transpose(1, 0, 2).reshape(D, H * DH)
        return (W.reshape(C, 128, D).astype(bf16),
                np.ascontiguousarray(be.reshape(H * DH).reshape(C, 128).T.astype(f32)))

    def pack_w(W, ncols):
        return W.reshape(-1, 128, ncols).astype(bf16)

    def pack_b(b):
        return np.ascontiguousarray(b.reshape(-1, 128).T.astype(f32))

    g1, b1 = inp["ln1_g"], inp["ln1_b"]
    g2, b2 = inp["ln2_g"], inp["ln2_b"]
    g3, b3 = inp["ln3_g"], inp["ln3_b"]

    wq1, bq1 = fold_qkv(inp["wq1"], g1, b1, sc)
    wk1, bk1 = fold_qkv(inp["wk1"], g1, b1, None)
    wv1, bv1 = fold_qkv(inp["wv1"], g1, b1, None)
    wq2, bq2 = fold_qkv(inp["wq2"], g2, b2, sc)
    wk2 = pack_w(inp["wk2"].transpose(1, 0, 2).reshape(D, H * DH), D)
    wv2 = pack_w(inp["wv2"].transpose(1, 0, 2).reshape(D, H * DH), D)

    wf1 = pack_w(inp["ffn_w1"] * g3[:, None], F)
    bf1 = pack_b(inp["ffn_b1"] + b3 @ inp["ffn_w1"])
    wf2 = pack_w(inp["ffn_w2"], D)

    tri = np.triu(np.ones((128, 128), f32))  # visible: t >= s
    m0 = np.concatenate([tri, np.ones((128, 128), f32)], 1)
    m1 = np.concatenate([np.zeros((128, 128), f32), tri], 1)

    shared = {
        "wq1": wq1, "wk1": wk1, "wv1": wv1, "wp1": pack_w(inp["proj1_w"], D),
        "wq2": wq2, "wk2": wk2, "wv2": wv2, "wp2": pack_w(inp["proj2_w"], D),
        "wf1": wf1, "wf2": wf2,
        "bq1": bq1, "bk1": bk1, "bv1": bv1, "bp1": pack_b(inp["proj1_b"]),
        "bq2": bq2, "bp2": pack_b(inp["proj2_b"]),
        "bf1": bf1, "bf2": pack_b(inp["ffn_b2"]),
        "mask0": np.concatenate([m0, m0], 1).astype(bf16),
        "mask1": np.concatenate([m1, m1], 1).astype(bf16),
    }

    def pack_act(a, dtype):
        # [BLOC, T, D] -> [PAIRS, C, 128, TP] feature-major
        a = a.reshape(PAIRS, 2, T, D).transpose(0, 3, 1, 2).reshape(PAIRS, D, TP)
        return np.ascontiguousarray(a.reshape(PAIRS, C, 128, TP).astype(dtype))

    in_maps = []
    for core in range(NCORES):
        s = slice(core * BLOC, (core + 1) * BLOC)
        m = dict(shared)
        m["x"] = pack_act(inp["x"][s], f32)
        m["enc"] = pack_act(inp["encoder_output"][s], bf16)
        in_maps.append(m)
    return in_maps


def _unpack_out(y):
    # [PAIRS, C, 128, TP] -> [BLOC, T, D]
    y = y.reshape(PAIRS, D, 2, T).transpose(0, 2, 3, 1)
    return y.reshape(BLOC, T, D)


TRACE = False
LAST_RESULTS = None


def kernel(**inputs):
    global LAST_RESULTS
    nc = _get_nc()
    in_maps = _prep_host(inputs)
    kw = {}
    if TRACE:
        kw = dict(trace=True, trace_cores=[0], stitch_traces=False)
    res = run_bass_kernel_spmd(nc, in_maps, core_ids=list(range(NCORES)), **kw)
    LAST_RESULTS = res
    out = np.concatenate([_unpack_out(r["y"]) for r in res.results], axis=0)
    return out.astype(np.float32)


if __name__ == "__main__":
    import reference
    inputs = {k: np.asarray(v) for k, v in reference.setup_inputs().items()}
    got = kernel(**inputs)
    want = np.asarray(reference.reference(**inputs))
    err = np.abs(got - want).max() / np.abs(want).max()
    print("Relative error:", err)

